# revision 12
# baseline (speedup 1.0000x reference)
"""Causal self-attention (RoPE, 16 heads, S=4096, D=1024) on 8 Trainium2 cores.

Sharding: tensor-parallel over heads — core c computes heads 2c, 2c+1.
Per core: q/k/v projections against its 128-row weight shard, transposed-score
attention (scores stored [k, q] so the softmax denominator folds into the PV
matmul via a ones-column on V), RoPE applied on-chip (pair-swap via SBUF-SBUF
DMAs + cos/sin elementwise ops), and a row-parallel output projection
producing a partial [S, D] result. Host sums the 8 partials.
Matmuls run in float32r (fast fp32 PE mode, ~5e-5 relative error).
"""
import sys
import numpy as np

sys.path.insert(0, "/opt/trn_rl_repo")

import concourse.bacc as bacc
import concourse.mybir as mybir
from concourse.tile import TileContext
from concourse.bass_utils import run_bass_kernel_spmd

FP = mybir.dt.float32
FR = mybir.dt.float32r

S = 4096          # sequence length
DM = 1024         # model dim
HD = 64           # head dim
NCORES = 8
ROPE_THETA = 10000.0
NQC = 8           # q chunks of 512
QW = 512
NKT = 32          # k tiles of 128
NDC = 8           # d-model chunks of 128

_CACHE = {}


def _build():
    nc = bacc.Bacc("TRN2", target_bir_lowering=False, debug=False,
                   num_devices=NCORES)

    xT = nc.dram_tensor("xT", [DM, S], FR, kind="ExternalInput")
    wq = nc.dram_tensor("wq", [DM, 128], FR, kind="ExternalInput")
    wk = nc.dram_tensor("wk", [DM, 128], FR, kind="ExternalInput")
    wv = nc.dram_tensor("wv", [DM, 128], FR, kind="ExternalInput")
    wo = nc.dram_tensor("wo", [128, DM], FR, kind="ExternalInput")
    cosm = nc.dram_tensor("cosm", [128, S], FP, kind="ExternalInput")
    sinm = nc.dram_tensor("sinm", [128, S], FP, kind="ExternalInput")
    ident = nc.dram_tensor("ident", [128, 128], FR, kind="ExternalInput")
    OUT = nc.dram_tensor("OUT", [S, DM], FP, kind="ExternalOutput")

    with nc.allow_low_precision(reason="float32r PE fast path"), \
         TileContext(nc) as tc:
        with tc.tile_pool(name="const", bufs=1) as cpool, \
             tc.tile_pool(name="big", bufs=1) as bpool, \
             tc.tile_pool(name="xt", bufs=12) as xpool, \
             tc.tile_pool(name="pt", bufs=2) as ptpool, \
             tc.tile_pool(name="work", bufs=2) as wpool, \
             tc.tile_pool(name="outp", bufs=2) as opool, \
             tc.tile_pool(name="ps", bufs=1, space="PSUM") as pspool:

            wq_sb = cpool.tile([128, DM], FR, tag="wq")
            wk_sb = cpool.tile([128, DM], FR, tag="wk")
            wv_sb = cpool.tile([128, DM], FR, tag="wv")
            wo_sb = cpool.tile([128, DM], FR, tag="wo")
            cos_sb = cpool.tile([128, S], FP, tag="cos")
            sin_sb = cpool.tile([128, S], FP, tag="sin")
            id_sb = cpool.tile([128, 128], FR, tag="ident")
            sel_sb = cpool.tile([1, 64], FR, tag="sel")

            # weight shards arrive as [DM, 128]; stage as [128, NDC*128] where
            # chunk dc holds rows dc*128..dc*128+127
            for w_sb, w_dr in ((wq_sb, wq), (wk_sb, wk), (wv_sb, wv)):
                nc.sync.dma_start(
                    w_sb[:].rearrange("p (c e) -> p c e", c=NDC),
                    w_dr[:].rearrange("(c p) e -> p c e", p=128))
            nc.sync.dma_start(wo_sb[:], wo[:])
            nc.sync.dma_start(cos_sb[:], cosm[:])
            nc.sync.dma_start(sin_sb[:], sinm[:])
            nc.sync.dma_start(id_sb[:], ident[:])
            nc.gpsimd.memset(sel_sb[:].bitcast(FP), 1.0)

            q_sb = bpool.tile([128, S], FR, tag="q")
            k_sb = bpool.tile([128, S], FR, tag="k")
            v_sb = bpool.tile([128, NKT, 130], FR, tag="v")
            o_sb = bpool.tile([128, S], FR, tag="o")

            # ones columns for the softmax-denominator rows of the PV matmuls
            nc.gpsimd.memset(v_sb[:, :, 64:65].bitcast(FP), 1.0)
            nc.gpsimd.memset(v_sb[:, :, 129:130].bitcast(FP), 1.0)

            # ---- projections: q/k/vT as [d, s] (head dims on partitions)
            for sc in range(NQC):
                ssl = slice(sc * QW, (sc + 1) * QW)
                xts = []
                for dc in range(NDC):
                    xt = xpool.tile([128, QW], FR, tag="xt")
                    nc.sync.dma_start(xt[:], xT[dc * 128:(dc + 1) * 128, ssl])
                    xts.append(xt)
                vt_tmp = wpool.tile([128, QW], FR, tag="vt")
                for w_sb, dst in ((wq_sb, q_sb[:, ssl]), (wk_sb, k_sb[:, ssl]),
                                  (wv_sb, vt_tmp[:])):
                    psp = pspool.tile([128, QW], FP, tag="mm", bufs=2)
                    for dc in range(NDC):
                        nc.tensor.matmul(psp[:], w_sb[:, dc * 128:(dc + 1) * 128],
                                         xts[dc][:], start=(dc == 0),
                                         stop=(dc == NDC - 1))
                    nc.vector.tensor_copy(dst, psp[:])
                # RoPE this chunk of q and k (interleaved-pair rotation)
                for t_sb in (q_sb, k_sb):
                    sw = wpool.tile([128, QW], FR, tag="sw")
                    nc.sync.dma_start(sw[0:128:2, :], t_sb[1:128:2, ssl])
                    nc.sync.dma_start(sw[1:128:2, :], t_sb[0:128:2, ssl])
                    t1 = wpool.tile([128, QW], FP, tag="t1")
                    t2 = wpool.tile([128, QW], FP, tag="t2")
                    nc.vector.tensor_tensor(t1[:], t_sb[:, ssl], cos_sb[:, ssl],
                                            mybir.AluOpType.mult)
                    nc.vector.tensor_tensor(t2[:], sw[:], sin_sb[:, ssl],
                                            mybir.AluOpType.mult)
                    nc.vector.tensor_tensor(t_sb[:, ssl], t1[:], t2[:],
                                            mybir.AluOpType.add)
                # transpose vT [d, s] -> v [s, d] per k-tile on the PE
                for j in range(4):
                    kt = 4 * sc + j
                    pst = pspool.tile([128, 128], FR, tag="mm", bufs=2)
                    nc.tensor.transpose(pst[:], vt_tmp[:, j * 128:(j + 1) * 128],
                                        id_sb[:])
                    nc.vector.tensor_copy(v_sb[:, kt, 0:64], pst[:, 0:64])
                    nc.vector.tensor_copy(v_sb[:, kt, 65:129], pst[:, 64:128])

            # ---- attention, transposed scores: sT[k, q] per head
            scale = 1.0 / np.sqrt(HD)
            for qc in range(NQC):
                qsl = slice(qc * QW, (qc + 1) * QW)
                nkt = 4 * (qc + 1)
                pv0 = pspool.tile([65, QW], FP, tag="pv0", bufs=1)
                pv1 = pspool.tile([65, QW], FP, tag="pv1", bufs=1)
                for g in range(nkt // 2):
                    ps_s = pspool.tile([128, 2048], FP, tag="s", bufs=1)
                    for j in range(2):
                        kt = 2 * g + j
                        ksl = slice(kt * 128, (kt + 1) * 128)
                        nc.tensor.matmul(ps_s[:, j * 512:(j + 1) * 512],
                                         k_sb[0:64, ksl], q_sb[0:64, qsl],
                                         start=True, stop=True,
                                         tile_position=(0, 0))
                        nc.tensor.matmul(ps_s[:, 1024 + j * 512:1024 + (j + 1) * 512],
                                         k_sb[64:128, ksl], q_sb[64:128, qsl],
                                         start=True, stop=True,
                                         tile_position=(64, 0))
                    pt = ptpool.tile([128, 2048], FR, tag="pt")
                    nc.scalar.activation(pt[:], ps_s[:],
                                         mybir.ActivationFunctionType.Exp,
                                         scale=scale)
                    for j in range(2):
                        kt = 2 * g + j
                        if kt >= 4 * qc:  # diagonal tile: zero where k > q
                            base = qc * QW - kt * 128
                            for off in (j * 512, 1024 + j * 512):
                                nc.gpsimd.affine_select(
                                    out=pt[:, off:off + 512],
                                    in_=pt[:, off:off + 512],
                                    compare_op=mybir.AluOpType.is_ge,
                                    fill=0.0, base=base,
                                    pattern=[[1, 512]], channel_multiplier=-1)
                    for j in range(2):
                        kt = 2 * g + j
                        nc.tensor.matmul(pv0[:], v_sb[:, kt, 0:65],
                                         pt[:, j * 512:(j + 1) * 512],
                                         start=(kt == 0), stop=(kt == nkt - 1))
                        nc.tensor.matmul(pv1[:], v_sb[:, kt, 65:130],
                                         pt[:, 1024 + j * 512:1024 + (j + 1) * 512],
                                         start=(kt == 0), stop=(kt == nkt - 1))

                # normalize: out rows / softmax denominator (row 64 of pv)
                r_sb = wpool.tile([1, 1024], FP, tag="r")
                nc.vector.reciprocal(r_sb[0:1, 0:512], pv0[64:65, :])
                nc.vector.reciprocal(r_sb[0:1, 512:1024], pv1[64:65, :])
                bcs = []
                for h in range(2):
                    bc = wpool.tile([64, QW], FP, tag="bc")
                    nc.gpsimd.partition_broadcast(
                        bc[:], r_sb[0:1, h * 512:(h + 1) * 512], channels=64)
                    bcs.append(bc)
                nc.vector.tensor_tensor(o_sb[0:64, qsl], pv0[0:64, :], bcs[0][:],
                                        mybir.AluOpType.mult)
                nc.vector.tensor_tensor(o_sb[64:128, qsl], pv1[0:64, :], bcs[1][:],
                                        mybir.AluOpType.mult)

                # final row-parallel projection for this q chunk
                for j2 in range(4):
                    st = qc * 4 + j2
                    ot = opool.tile([128, DM], FP, tag="ot")
                    for eh in range(2):
                        pf = pspool.tile([128, QW], FP, tag="mm", bufs=2)
                        nc.tensor.matmul(pf[:], o_sb[:, st * 128:(st + 1) * 128],
                                         wo_sb[:, eh * 512:(eh + 1) * 512],
                                         start=True, stop=True)
                        nc.vector.tensor_copy(ot[:, eh * 512:(eh + 1) * 512], pf[:])
                    nc.sync.dma_start(OUT[st * 128:(st + 1) * 128, :], ot[:])

    nc.compile()
    return nc


def _host_prep(x, Wq, Wk, Wv, Wo):
    x = np.asarray(x, dtype=np.float32)
    Wq = np.asarray(Wq, dtype=np.float32)
    Wk = np.asarray(Wk, dtype=np.float32)
    Wv = np.asarray(Wv, dtype=np.float32)
    Wo = np.asarray(Wo, dtype=np.float32)

    xT = np.ascontiguousarray(x.reshape(S, DM).T)

    # RoPE tables in the [d, s] layout (fp32 math to match the reference)
    pos = np.arange(S, dtype=np.float32)
    inv_freq = (ROPE_THETA ** (-np.arange(0, HD, 2, dtype=np.float32) / HD))
    ang = pos[None, :] * inv_freq[:, None]          # [32, S]
    cos_p = np.cos(ang).astype(np.float32)
    sin_p = np.sin(ang).astype(np.float32)
    cosm = np.empty((128, S), np.float32)
    sinm = np.empty((128, S), np.float32)
    for h in range(2):
        b = h * HD
        cosm[b + 0:b + HD:2] = cos_p
        cosm[b + 1:b + HD:2] = cos_p
        sinm[b + 0:b + HD:2] = -sin_p
        sinm[b + 1:b + HD:2] = sin_p

    ident = np.eye(128, dtype=np.float32)

    in_maps = []
    for c in range(NCORES):
        rows = slice(128 * c, 128 * (c + 1))
        in_maps.append({
            "xT": xT,
            "wq": np.ascontiguousarray(Wq[rows, :].T),
            "wk": np.ascontiguousarray(Wk[rows, :].T),
            "wv": np.ascontiguousarray(Wv[rows, :].T),
            "wo": np.ascontiguousarray(Wo[:, rows].T),
            "cosm": cosm,
            "sinm": sinm,
            "ident": ident,
        })
    return in_maps


def kernel(x, Wq, Wk, Wv, Wo, _trace=False, _trace_kwargs=None):
    if "nc" not in _CACHE:
        _CACHE["nc"] = _build()
    nc = _CACHE["nc"]
    in_maps = _host_prep(x, Wq, Wk, Wv, Wo)
    kw = {}
    if _trace:
        kw = dict(trace=True, **(_trace_kwargs or {}))
    res = run_bass_kernel_spmd(nc, in_maps, core_ids=list(range(NCORES)), **kw)
    out = np.zeros((S, DM), np.float64)
    for r in res.results:
        out += np.asarray(r["OUT"], dtype=np.float64)
    _CACHE["last_results"] = res
    return out.astype(np.float32).reshape(1, S, DM)
